# revision 12
# baseline (speedup 1.0000x reference)
"""MLA q/k/v projection kernel for Trainium2 (8 NeuronCores, token-data-parallel).

Self-contained: hardcodes the problem shapes from nn_MLA_81106162418389.
  hidden_state [2, 4096, 2048] f32 -> out [2, 16, 4096, 512] f32
Strategy: shard the 8192 tokens over 8 cores (1024 each); replicate weights.
Matmuls run in fp32r (tf32) mode; data pre-rounded to tf32 on host.
"""
import sys
sys.path.insert(0, "/opt/trn_rl_repo")

import numpy as np

import concourse.bass as bass
import concourse.tile as tile
from concourse import bacc, mybir
from concourse import bass2jax
from concourse.masks import make_identity

# ---- problem constants ----
HID, QK_NOPE, QK_ROPE, Q_LR, KV_LR, H, V_DIM = 2048, 128, 64, 768, 512, 16, 128
QK_HEAD = QK_NOPE + QK_ROPE           # 192
OUT_C = 2 * QK_HEAD + V_DIM           # 512
B, S = 2, 4096
THETA = 10000.0
EPS = 1e-5

N_CORES = 8
T = (B * S) // N_CORES                # 1024 tokens per core
P = 128
TCN = T // P                          # 8 token chunks
NT = 2                                # 512-wide token tiles for phase 1
KO = HID // P                         # 16 k-chunks for a-proj
ROQ = Q_LR // P                       # 6 r-chunks for q up-proj
ROKV = KV_LR // P                     # 4 r-chunks for kv up-proj
HH = H // 2                           # 8 heads per half-pass

F32 = mybir.dt.float32
R32 = mybir.dt.float32r
AF = mybir.ActivationFunctionType
OP = mybir.AluOpType


def _build(n_repeats=1, has_qb_bias=False, has_kvb_bias=False):
    nc = bacc.Bacc("TRN2", target_bir_lowering=False, debug=False,
                   num_devices=N_CORES)

    xT_d = nc.dram_tensor("xT", [HID, T], R32, kind="ExternalInput").ap()
    wqaT_d = nc.dram_tensor("wqaT", [HID, Q_LR], R32, kind="ExternalInput").ap()
    wkvaT_d = nc.dram_tensor("wkvaT", [HID, KV_LR + QK_ROPE], R32,
                             kind="ExternalInput").ap()
    wqbT_d = nc.dram_tensor("wqbT", [Q_LR, H * QK_HEAD], R32,
                            kind="ExternalInput").ap()
    wkvbT_d = nc.dram_tensor("wkvbT", [KV_LR, H * (QK_NOPE + V_DIM)], R32,
                             kind="ExternalInput").ap()
    bqa_d = nc.dram_tensor("bqa", [Q_LR], F32, kind="ExternalInput").ap()
    bkva_d = nc.dram_tensor("bkvap", [5 * P], F32, kind="ExternalInput").ap()
    cos_d = nc.dram_tensor("cosb", [T, QK_ROPE], F32, kind="ExternalInput").ap()
    sin_d = nc.dram_tensor("sinb", [T, QK_ROPE], F32, kind="ExternalInput").ap()
    if has_qb_bias:
        bqb_d = nc.dram_tensor("bqb", [H * QK_HEAD], F32, kind="ExternalInput").ap()
    if has_kvb_bias:
        bkvb_d = nc.dram_tensor("bkvb", [H * (QK_NOPE + V_DIM)], F32,
                                kind="ExternalInput").ap()
    ones_d = nc.dram_tensor("ones", [P, 1], R32, kind="ExternalInput").ap()
    out_d = nc.dram_tensor("out", [H, T, OUT_C], F32, kind="ExternalOutput").ap()

    for _ in range(n_repeats):
        _emit_once(nc, xT_d, wqaT_d, wkvaT_d, wqbT_d, wkvbT_d, bqa_d, bkva_d,
                   cos_d, sin_d,
                   bqb_d if has_qb_bias else None,
                   bkvb_d if has_kvb_bias else None,
                   out_d, ones_d)
    nc.compile()
    return nc


def _emit_once(nc, xT_d, wqaT_d, wkvaT_d, wqbT_d, wkvbT_d, bqa_d, bkva_d,
               cos_d, sin_d, bqb_d, bkvb_d, out_d, ones_d):
    KVC = ROKV + 1          # 5 feature chunks for kv a-proj (last is 64-wide rope)
    with tile.TileContext(nc) as tc:
        with tc.tile_pool(name="persist", bufs=1) as pp, \
             tc.tile_pool(name="acts", bufs=1) as ap_pool:

            # ---- small persistent tiles ----
            cos_sb = pp.tile([P, TCN, QK_ROPE], F32)
            nc.sync.dma_start(cos_sb[:], cos_d.rearrange("(tc p) c -> p tc c", p=P))
            sin_sb = pp.tile([P, TCN, QK_ROPE], F32)
            nc.sync.dma_start(sin_sb[:], sin_d.rearrange("(tc p) c -> p tc c", p=P))
            bqa_sb = pp.tile([P, ROQ], F32)
            nc.sync.dma_start(bqa_sb[:], bqa_d.rearrange("(c p) -> p c", p=P))
            bkva_sb = pp.tile([P, 5], F32)
            nc.sync.dma_start(bkva_sb[:], bkva_d.rearrange("(c p) -> p c", p=P))
            ones_r = pp.tile([P, 1], R32)
            nc.sync.dma_start(ones_r[:], ones_d[:])
            eps_t = pp.tile([1, 1], F32)
            nc.gpsimd.memset(eps_t[:], EPS)
            ident = pp.tile([P, P], F32)
            make_identity(nc, ident[:])

            # activations (live across both phases)
            q_cT = ap_pool.tile([P, ROQ, T], R32)       # q low-rank, [feat, tok]
            kv_cT = ap_pool.tile([P, ROKV, T], R32)     # kv low-rank, [feat, tok]
            krot = ap_pool.tile([P, TCN, QK_ROPE], F32)  # roped k, [tok, chan]

            # ================= phase 1: a-projections + LN =================
            with tc.tile_pool(name="xw", bufs=1) as xw, \
                 tc.tile_pool(name="stat_sb", bufs=1) as st, \
                 tc.tile_pool(name="sq", bufs=3) as sqp, \
                 tc.tile_pool(name="ps1", bufs=3, space="PSUM") as ps1, \
                 tc.tile_pool(name="pst", bufs=3, space="PSUM") as pst, \
                 tc.tile_pool(name="ptr", bufs=1, space="PSUM") as ptr:

                krope = st.tile([64, T], F32)     # raw k rope slice [chan, tok]
                istd_bc = st.tile([P, T], F32, tag="istdbc", bufs=1)
                nmi_bc = st.tile([P, T], F32, tag="nmibc", bufs=1)

                x_t = []
                wa_t = []
                for k in range(KO):
                    xt = xw.tile([P, T], R32, tag="x", bufs=KO)
                    nc.sync.dma_start(xt[:], xT_d[k * P:(k + 1) * P, :])
                    x_t.append(xt)
                    wt = xw.tile([P, Q_LR], R32, tag="wa", bufs=KO)
                    nc.sync.dma_start(wt[:], wqaT_d[k * P:(k + 1) * P, :])
                    wa_t.append(wt)

                # ---- mm1 q: q_cT[f, t] ----
                for nt in range(NT):
                    ts = slice(nt * 512, (nt + 1) * 512)
                    for g in range(2):          # groups of 3 psums
                        pss = [ps1.tile([P, 512], F32, name=f"ps1_{nt}_{g}_{fi}", tag="ps1", bufs=3) for fi in range(3)]
                        for k in range(KO):
                            for fi in range(3):
                                fc = g * 3 + fi
                                nc.tensor.matmul(
                                    pss[fi][:], wa_t[k][:, fc * P:(fc + 1) * P],
                                    x_t[k][:, ts],
                                    start=(k == 0), stop=(k == KO - 1))
                        for fi in range(3):
                            fc = g * 3 + fi
                            nc.scalar.activation(q_cT[:, fc, ts], pss[fi][:],
                                                 AF.Identity,
                                                 bias=bqa_sb[:, fc:fc + 1])

                # ---- mm1 kv (reuse "wa" slots) ----
                wkv_t = []
                for k in range(KO):
                    wt = xw.tile([P, KV_LR + QK_ROPE], R32, tag="wa", bufs=KO)
                    nc.sync.dma_start(wt[:], wkvaT_d[k * P:(k + 1) * P, :])
                    wkv_t.append(wt)
                for nt in range(NT):
                    ts = slice(nt * 512, (nt + 1) * 512)
                    for g, nfc in ((0, 3), (1, 2)):
                        pss = [ps1.tile([P, 512], F32, name=f"ps1kv_{nt}_{g}_{fi}", tag="ps1", bufs=3) for fi in range(nfc)]
                        for k in range(KO):
                            for fi in range(nfc):
                                fc = g * 3 + fi
                                w = 64 if fc == 4 else P
                                nc.tensor.matmul(
                                    pss[fi][:w],
                                    wkv_t[k][:, fc * P:fc * P + w],
                                    x_t[k][:, ts],
                                    start=(k == 0), stop=(k == KO - 1))
                        for fi in range(nfc):
                            fc = g * 3 + fi
                            if fc == 4:
                                nc.scalar.activation(krope[:, ts], pss[fi][:64],
                                                     AF.Identity,
                                                     bias=bkva_sb[:64, 4:5])
                            else:
                                nc.scalar.activation(kv_cT[:, fc, ts], pss[fi][:],
                                                     AF.Identity,
                                                     bias=bkva_sb[:, fc:fc + 1])

                # ---- LN stats (q then kv) ----
                for which, src, nfc, dim in (
                        ("q", q_cT, ROQ, Q_LR),
                        ("kv", kv_cT, ROKV, KV_LR)):
                    for nt in range(NT):
                        ts = slice(nt * 512, (nt + 1) * 512)
                        ps_s = pst.tile([1, 512], F32, name=f"pss_{which}{nt}",
                                        tag="pst", bufs=3)
                        for fc in range(nfc):
                            nc.tensor.matmul(ps_s[:], ones_r[:], src[:, fc, ts],
                                             start=(fc == 0), stop=(fc == nfc - 1))
                        ps_q = pst.tile([1, 512], F32, name=f"psq_{which}{nt}",
                                        tag="pst", bufs=3)
                        for fc in range(nfc):
                            sq = sqp.tile([P, 512], R32, tag="sq")
                            nc.scalar.activation(sq[:], src[:, fc, ts], AF.Square)
                            nc.tensor.matmul(ps_q[:], ones_r[:], sq[:],
                                             start=(fc == 0), stop=(fc == nfc - 1))
                        mu = st.tile([1, 512], F32, tag="mu", bufs=2,
                                     name=f"mu_{which}{nt}")
                        istd = st.tile([1, 512], F32, tag="istd", bufs=2,
                                       name=f"istd_{which}{nt}")
                        nmi = st.tile([1, 512], F32, tag="nmi", bufs=2,
                                      name=f"nmi_{which}{nt}")
                        nc.scalar.mul(mu[:], ps_s[:], 1.0 / dim)
                        # istd <- mean(x^2) - mu^2
                        nc.vector.tensor_tensor(istd[:], mu[:], mu[:], OP.mult)
                        nc.vector.scalar_tensor_tensor(
                            istd[:], ps_q[:], 1.0 / dim, istd[:],
                            OP.mult, OP.subtract)
                        nc.scalar.activation(istd[:], istd[:], AF.Sqrt,
                                             bias=eps_t[:, 0:1])
                        nc.vector.reciprocal(istd[:], istd[:])
                        nc.vector.scalar_tensor_tensor(
                            nmi[:], mu[:], -1.0, istd[:], OP.mult, OP.mult)
                        nc.gpsimd.partition_broadcast(istd_bc[:, ts], istd[:])
                        nc.gpsimd.partition_broadcast(nmi_bc[:, ts], nmi[:])
                    # normalize in place (writes tf32)
                    for fc in range(nfc):
                        nc.vector.tensor_tensor(src[:, fc, :], src[:, fc, :],
                                                istd_bc[:], OP.mult)
                        nc.vector.tensor_tensor(src[:, fc, :], src[:, fc, :],
                                                nmi_bc[:], OP.add)

                # ---- k rope: transpose [64, T] -> [T, 64], then rotate ----
                krope_t = st.tile([P, TCN, QK_ROPE], F32)
                for tci in range(TCN):
                    ps_t = ptr.tile([P, 64], F32)
                    nc.tensor.transpose(ps_t[:], krope[:, tci * P:(tci + 1) * P],
                                        ident[:64, :64])
                    nc.scalar.copy(krope_t[:, tci, :], ps_t[:])
                tmp = st.tile([P, TCN, 32], F32)
                tmp2 = st.tile([P, TCN, 32], F32)
                nc.vector.tensor_tensor(tmp[:], krope_t[:, :, 32:64],
                                        sin_sb[:, :, 0:32], OP.mult)
                nc.vector.tensor_tensor(tmp2[:], krope_t[:, :, 0:32],
                                        sin_sb[:, :, 32:64], OP.mult)
                nc.vector.tensor_tensor(krot[:], krope_t[:], cos_sb[:], OP.mult)
                nc.vector.tensor_tensor(krot[:, :, 0:32], krot[:, :, 0:32],
                                        tmp[:], OP.subtract)
                nc.vector.tensor_tensor(krot[:, :, 32:64], krot[:, :, 32:64],
                                        tmp2[:], OP.add)

            # ================= phase 2: up-projections + assemble =============
            with tc.tile_pool(name="wb", bufs=1) as wb, \
                 tc.tile_pool(name="outp", bufs=2) as outp, \
                 tc.tile_pool(name="rsc", bufs=3) as rsc, \
                 tc.tile_pool(name="bias2", bufs=1) as bias2, \
                 tc.tile_pool(name="ps2", bufs=2, space="PSUM") as ps2:

                QW = HH * QK_HEAD        # 1536 q cols per half
                KW = HH * (QK_NOPE + V_DIM)  # 2048 kv cols per half

                bqb_bc = bkvb_bc = None
                if bqb_d is not None:
                    b1 = bias2.tile([1, H * QK_HEAD], F32)
                    nc.sync.dma_start(b1[:], bqb_d[None, :])
                    bqb_bc = bias2.tile([P, H * QK_HEAD], F32)
                    nc.gpsimd.partition_broadcast(bqb_bc[:], b1[:])
                if bkvb_d is not None:
                    b2 = bias2.tile([1, H * (QK_NOPE + V_DIM)], F32)
                    nc.sync.dma_start(b2[:], bkvb_d[None, :])
                    bkvb_bc = bias2.tile([P, H * (QK_NOPE + V_DIM)], F32)
                    nc.gpsimd.partition_broadcast(bkvb_bc[:], b2[:])

                for half in range(2):
                    h0 = half * HH
                    c0q = h0 * QK_HEAD
                    c0kv = h0 * (QK_NOPE + V_DIM)
                    wq_t = []
                    for ro in range(ROQ):
                        wt = wb.tile([P, QW], R32, tag="wq", bufs=ROQ + 2)
                        nc.sync.dma_start(
                            wt[:], wqbT_d[ro * P:(ro + 1) * P, c0q:c0q + QW])
                        wq_t.append(wt)
                    wkv_t = []
                    for ro in range(ROKV):
                        wt = wb.tile([P, KW], R32, tag="wkv", bufs=ROKV + 1)
                        nc.sync.dma_start(
                            wt[:], wkvbT_d[ro * P:(ro + 1) * P, c0kv:c0kv + KW])
                        wkv_t.append(wt)

                    for tci in range(TCN):
                        tsl = slice(tci * P, (tci + 1) * P)
                        ob = outp.tile([P, HH, OUT_C], F32)
                        obv = ob.rearrange("p (i j) c -> p i j c", j=2)

                        # ---- q up-proj: 4 psum slots x 384 (2 heads each) ----
                        psq = ps2.tile([P, 4, 512], F32, name=f"psq_{half}_{tci}", tag="ps2", bufs=2)
                        for i in range(4):
                            for ro in range(ROQ):
                                nc.tensor.matmul(
                                    psq[:, i, 0:384], q_cT[:, ro, tsl],
                                    wq_t[ro][:, i * 384:(i + 1) * 384],
                                    start=(ro == 0), stop=(ro == ROQ - 1))
                        if bqb_bc is not None:
                            nc.vector.tensor_tensor(
                                psq[:, :, 0:384], psq[:, :, 0:384],
                                bqb_bc[:, c0q:c0q + QW].rearrange(
                                    "p (i c) -> p i c", c=384), OP.add)
                        # q_nope copies
                        for j in range(2):
                            nc.scalar.copy(obv[:, :, j, 0:QK_NOPE],
                                           psq[:, :, j * QK_HEAD:
                                               j * QK_HEAD + QK_NOPE])
                        # q rope (batched over i and j)
                        cosb = cos_sb[:, tci:tci + 1, None, :].to_broadcast(
                            [P, 4, 2, QK_ROPE])
                        sinb = sin_sb[:, tci:tci + 1, None, :].to_broadcast(
                            [P, 4, 2, QK_ROPE])
                        xr = psq[:, :, 0:2 * QK_HEAD].rearrange(
                            "p i (j c) -> p i j c", c=QK_HEAD)[
                            :, :, :, QK_NOPE:QK_HEAD]
                        orp = obv[:, :, :, QK_NOPE:QK_HEAD]
                        t1 = rsc.tile([P, 4, 2, 32], F32, tag="t1")
                        t2 = rsc.tile([P, 4, 2, 32], F32, tag="t2")
                        nc.vector.tensor_tensor(t1[:], xr[:, :, :, 32:64],
                                                sinb[:, :, :, 0:32], OP.mult)
                        nc.vector.tensor_tensor(t2[:], xr[:, :, :, 0:32],
                                                sinb[:, :, :, 32:64], OP.mult)
                        nc.vector.tensor_tensor(orp[:], xr[:], cosb[:], OP.mult)
                        nc.vector.tensor_tensor(orp[:, :, :, 0:32],
                                                orp[:, :, :, 0:32], t1[:],
                                                OP.subtract)
                        nc.vector.tensor_tensor(orp[:, :, :, 32:64],
                                                orp[:, :, :, 32:64], t2[:],
                                                OP.add)

                        # ---- kv up-proj: 4 psum slots x 512 (2 heads each) ----
                        pskv = ps2.tile([P, 4, 512], F32, name=f"pskv_{half}_{tci}", tag="ps2", bufs=2)
                        for i in range(4):
                            for ro in range(ROKV):
                                nc.tensor.matmul(
                                    pskv[:, i, :], kv_cT[:, ro, tsl],
                                    wkv_t[ro][:, i * 512:(i + 1) * 512],
                                    start=(ro == 0), stop=(ro == ROKV - 1))
                        if bkvb_bc is not None:
                            nc.vector.tensor_tensor(
                                pskv[:], pskv[:],
                                bkvb_bc[:, c0kv:c0kv + KW].rearrange(
                                    "p (i c) -> p i c", c=512), OP.add)
                        for j in range(2):
                            # k_nope -> out cols 192:320
                            nc.scalar.copy(
                                obv[:, :, j, QK_HEAD:QK_HEAD + QK_NOPE],
                                pskv[:, :, j * 256:j * 256 + QK_NOPE])
                            # v -> out cols 384:512
                            nc.scalar.copy(
                                obv[:, :, j, 2 * QK_HEAD:OUT_C],
                                pskv[:, :, j * 256 + QK_NOPE:(j + 1) * 256])
                        # k_rot broadcast -> out cols 320:384
                        nc.scalar.copy(
                            ob[:, :, QK_HEAD + QK_NOPE:2 * QK_HEAD],
                            krot[:, tci:tci + 1, :].to_broadcast(
                                [P, HH, QK_ROPE]))

                        nc.sync.dma_start(
                            out_d.rearrange("h t c -> t h c")[
                                tsl, h0:h0 + HH, :], ob[:])


# ------------------------- host side -------------------------

def _round_tf32(x):
    u = np.ascontiguousarray(x).view(np.uint32).astype(np.uint64)
    u = (u + 0xFFF + ((u >> 13) & 1)) >> 13 << 13
    return (u & 0xFFFFFFFF).astype(np.uint32).view(np.float32)


def _rope_tables(s0):
    pos = np.arange(s0, s0 + T, dtype=np.float64)
    inv = 1.0 / THETA ** (np.arange(0, QK_ROPE, 2, dtype=np.float64) / QK_ROPE)
    fr = pos[:, None] * inv[None, :]
    cos = np.concatenate([np.cos(fr), np.cos(fr)], axis=1).astype(np.float32)
    sin = np.concatenate([np.sin(fr), np.sin(fr)], axis=1).astype(np.float32)
    return cos, sin


_prog_cache = {}


def kernel(hidden_state, w_qa, b_qa, g_qa_ln, b_qa_ln, w_qb, b_qb,
           w_kva, b_kva, g_kva_ln, b_kva_ln, w_kvb, b_kvb):
    hidden_state = np.asarray(hidden_state, dtype=np.float32)
    w_qa = np.asarray(w_qa, dtype=np.float32)
    w_qb = np.asarray(w_qb, dtype=np.float32)
    w_kva = np.asarray(w_kva, dtype=np.float32)
    w_kvb = np.asarray(w_kvb, dtype=np.float32)
    b_qa = np.asarray(b_qa, dtype=np.float32)
    b_kva = np.asarray(b_kva, dtype=np.float32)
    g_qa_ln = np.asarray(g_qa_ln, dtype=np.float32)
    b_qa_ln = np.asarray(b_qa_ln, dtype=np.float32)
    g_kva_ln = np.asarray(g_kva_ln, dtype=np.float32)
    b_kva_ln = np.asarray(b_kva_ln, dtype=np.float32)
    b_qb = np.asarray(b_qb, dtype=np.float32)
    b_kvb = np.asarray(b_kvb, dtype=np.float32)

    wqaT = _round_tf32(w_qa.T)
    wkvaT = _round_tf32(w_kva.T)
    wqbT = _round_tf32((w_qb * g_qa_ln[None, :]).T)
    wkvbT = _round_tf32((w_kvb * g_kva_ln[None, :]).T)
    bqb_eff = (b_qb + w_qb @ b_qa_ln).astype(np.float32)
    bkvb_eff = (b_kvb + w_kvb @ b_kva_ln).astype(np.float32)
    bkva_pad = np.zeros(5 * P, np.float32)
    bkva_pad[:KV_LR + QK_ROPE] = b_kva

    has_qb = bool(np.any(bqb_eff))
    has_kvb = bool(np.any(bkvb_eff))
    key = (has_qb, has_kvb)
    if key not in _prog_cache:
        _prog_cache[key] = _build(1, has_qb, has_kvb)
    nc = _prog_cache[key]

    flat = hidden_state.reshape(B * S, HID)
    in_maps = []
    for c in range(N_CORES):
        tok0 = c * T
        s0 = tok0 % S
        cos, sin = _rope_tables(s0)
        m = {
            "xT": _round_tf32(flat[tok0:tok0 + T].T),
            "wqaT": wqaT, "wkvaT": wkvaT, "wqbT": wqbT, "wkvbT": wkvbT,
            "bqa": b_qa, "bkvap": bkva_pad, "cosb": cos, "sinb": sin,
            "ones": np.ones((P, 1), np.float32),
        }
        if has_qb:
            m["bqb"] = bqb_eff
        if has_kvb:
            m["bkvb"] = bkvb_eff
        in_maps.append(m)

    res = bass2jax.run_bass_via_pjrt(nc, in_maps, n_cores=N_CORES)

    out = np.empty((B, H, S, OUT_C), np.float32)
    for c in range(N_CORES):
        tok0 = c * T
        b = tok0 // S
        s0 = tok0 % S
        out[b, :, s0:s0 + T, :] = res[c]["out"]
    return out
